# revision 14
# baseline (speedup 1.0000x reference)
"""Causal dot-product attention (B=4, S=2048, D=1024) on 8 TRN2 NeuronCores.

v3: bf16 inputs (kT/qT/vT) + bf16 Wv with rank-1 bf16-rounding correction for
K/Q projections (Wk/Wq stay f32r); phase order V->K->Q so the startup
transient needs only ~1.5MB before the PE saturates; PE p-state warm-up
matmuls during the DMA fill; mask via a per-core [128,256] additive tile
fused into the last logits block (no 2MB mask DMA); probs transposed on the
DMA xbar for big slots / PE for small slots; attention slots in descending
extent order.

Sharding: batch x query-tile-class as v2. Core c handles batch c//2; slot s
of class 0 gets tile 15-2s (extent 16-2s chunks), class 1 tile 14-2s
(true extent 15-2s, padded to 16-2s; the pad + diagonal are masked by the
MB tile data, keeping one SPMD program).

Numerics: projections contract bf16 inputs against f32r (K/Q) or bf16 (V)
weights at full PE speed; the dominant rank-1 rounding error
rowsum(x - bf16(x)) (x) colmean(W~) is corrected during PSUM evacuation
(fused scalar_tensor_tensor). QK runs f32r q1/k1. 1/sqrt(D) is applied in
the exp activation; logits' last 256 columns get the additive mask during
evacuation.
"""
import numpy as np
import ml_dtypes
import concourse.bass as bass
import concourse.mybir as mybir
from concourse import bacc
from concourse.tile import TileContext
from concourse.bass_utils import run_bass_kernel_spmd
from concourse.masks import make_identity

f32 = mybir.dt.float32
f32r = mybir.dt.float32r
bf16 = mybir.dt.bfloat16
fp16 = mybir.dt.float16
AF = mybir.ActivationFunctionType
ALU = mybir.AluOpType

B, S, D = 4, 2048, 1024
SH = 1024                  # query rows per core
EXT = [16, 14, 12, 10, 8, 6, 4, 2]        # key extent per slot, 128-chunks
NB512 = [e // 4 for e in EXT]             # full 512 blocks
NB256 = [(e % 4) // 2 for e in EXT]       # trailing 256 block (0 or 1)
TILES = [[15, 13, 11, 9, 7, 5, 3, 1], [14, 12, 10, 8, 6, 4, 2, 0]]
SCALE = 1.0 / 32.0
NEG = -float(2 ** 30)
NWARM = 16


def rne11(x):
    """Bit-exact f32r rounding: RNE to 11 mantissa bits."""
    b = np.asarray(x, dtype=np.float32).view(np.uint32).astype(np.uint64)
    half = np.uint64(1 << 11)
    lsb = (b >> np.uint64(12)) & np.uint64(1)
    b2 = ((b + half - np.uint64(1) + lsb) >> np.uint64(12)) << np.uint64(12)
    return b2.astype(np.uint32).view(np.float32)


def bf16r(x):
    """Round f32 -> bf16 grid (RNE), returned as f32."""
    return np.asarray(x, dtype=np.float32).astype(ml_dtypes.bfloat16).astype(np.float32)


def relay(xT, nsb):
    """[D, N] -> [nsb, 128, 8, N//nsb] per-partition-contiguous, bf16."""
    Dn, N = xT.shape
    w = N // nsb
    out = np.asarray(xT, np.float32).reshape(8, 128, nsb, w).transpose(2, 1, 0, 3)
    return np.ascontiguousarray(out.astype(ml_dtypes.bfloat16))


def relay_w(W):
    """[D, D] fp16 -> [2, 128, 4, 8, 128]: [half][p][d4][din][dout]."""
    out = np.asarray(W, np.float32).reshape(8, 128, 2, 4, 128).transpose(2, 1, 3, 0, 4)
    return np.ascontiguousarray(out.astype(np.float16))


def relay_wv(W):
    """[D, D] bf16 -> [2, 128, 8, 4, 128]: [half][p][din][d4][dout]."""
    out = np.asarray(W, np.float32).reshape(8, 128, 2, 4, 128).transpose(2, 1, 0, 3, 4)
    return np.ascontiguousarray(out.astype(ml_dtypes.bfloat16))


def build():
    nc = bacc.Bacc()
    qT = nc.dram_tensor("qT", [2, 128, 8, 512], bf16, kind="ExternalInput")
    kT = nc.dram_tensor("kT", [4, 128, 8, 512], bf16, kind="ExternalInput")
    vT = nc.dram_tensor("vT", [4, 128, 8, 512], bf16, kind="ExternalInput")
    Wq = nc.dram_tensor("Wq", [2, 128, 4, 8, 128], fp16, kind="ExternalInput")
    Wk = nc.dram_tensor("Wk", [2, 128, 4, 8, 128], fp16, kind="ExternalInput")
    Wv = nc.dram_tensor("Wv", [2, 128, 8, 4, 128], bf16, kind="ExternalInput")
    MB = nc.dram_tensor("MB", [128, 256], f32, kind="ExternalInput")
    DK = nc.dram_tensor("DK", [128, S], f32, kind="ExternalInput")
    DQ = nc.dram_tensor("DQ", [128, SH], f32, kind="ExternalInput")
    CK = nc.dram_tensor("CK", [128, 8], f32, kind="ExternalInput")
    CQ = nc.dram_tensor("CQ", [128, 8], f32, kind="ExternalInput")
    O = nc.dram_tensor("O", [SH, D], f32, kind="ExternalOutput")

    with TileContext(nc) as tc:
        with tc.tile_pool(name="pers", bufs=1) as pers:
            k1T = pers.tile([128, 8, S], f32r, tag="k1T")      # 64 KB/part
            v1 = pers.tile([128, 16, D], bf16, tag="v1")       # 32 KB/part
            q1T = pers.tile([128, 8, SH], f32r, tag="q1T")     # 32 KB/part

            consts = tc.alloc_tile_pool(name="consts", bufs=1, side="left")
            warm_p = tc.alloc_tile_pool(name="warm", bufs=1, side="left")
            corrK = tc.alloc_tile_pool(name="corrK", bufs=1, side="left")
            inp = tc.alloc_tile_pool(name="inp", bufs=3, side="left")
            wvp = tc.alloc_tile_pool(name="wvp", bufs=1, side="right")
            pps = tc.alloc_tile_pool(name="pps", bufs=8, space="PSUM")

            ident = consts.tile([128, 128], bf16, tag="ident")
            mb = consts.tile([128, 256], f32, tag="mb")
            ck = consts.tile([128, 8], f32, tag="ck")
            cq = consts.tile([128, 8], f32, tag="cq")
            dkb = corrK.tile([128, S], f32, tag="dkb")          # 8 KB/part
            dqb = consts.tile([128, SH], f32, tag="dqb")        # 4 KB/part

            # ---- PE p-state warm-up: run scratch matmuls while DMAs fill ----
            scr = warm_p.tile([128, 512], bf16, tag="scr")
            nc.gpsimd.memset(scr[:], 0.0)
            wps = pps.tile([128, 512], f32, tag="pp")
            for _ in range(NWARM):
                nc.tensor.matmul(wps[:, 0:256], scr[:, 0:128], scr[:, 0:256],
                                 start=True, stop=True)

            # =============== phase V: v1 = vT^T Wv (bf16, no correction) ===========
            # startup-critical stream: wv half-pieces on gpsimd, vT0 on sync
            wv0 = wvp.tile([128, 8, 4, 128], bf16, tag="wv0")
            wv1 = wvp.tile([128, 8, 4, 128], bf16, tag="wv1")
            it0 = inp.tile([128, 8, 512], bf16, tag="inT")
            # single sync-queue stream in consumption order: one queue with
            # large pieces sustains ~335GB/s; splitting across queues is slower
            nc.sync.dma_start(out=wv0[:, 0:2], in_=Wv[0, :, 0:2])
            nc.sync.dma_start(out=it0[:, 0:2, :], in_=vT[0, :, 0:2])
            nc.sync.dma_start(out=wv0[:, 2:8], in_=Wv[0, :, 2:8])
            nc.sync.dma_start(out=it0[:, 2:8, :], in_=vT[0, :, 2:8])
            nc.sync.dma_start(out=wv1[:], in_=Wv[1])
            nc.scalar.dma_start(out=ck[:], in_=CK[:, :])
            nc.scalar.dma_start(out=cq[:], in_=CQ[:, :])
            nc.scalar.dma_start(out=mb[:], in_=MB[:, :])
            make_identity(nc, ident[:])
            wk = [None, None]
            wq = [None, None]
            wkp = tc.alloc_tile_pool(name="wkp", bufs=1, side="left")

            its = [it0, None, None, None]
            kits = [None, None, None, None]
            for sb in range(4):
                it = its[sb]
                # prefetch on the single sync stream, consumption order
                if sb < 3:
                    nx = inp.tile([128, 8, 512], bf16, tag="inT", name=f"vin{sb+1}")
                    nc.sync.dma_start(out=nx[:], in_=vT[sb + 1])
                    its[sb + 1] = nx
                if sb == 1:
                    wk0t = wkp.tile([128, 4, 8, 128], fp16, tag="wk0")
                    nc.sync.dma_start(out=wk0t[:], in_=Wk[0])
                    wk[0] = wk0t
                if sb == 2:
                    wk1t = wkp.tile([128, 4, 8, 128], fp16, tag="wk1")
                    nc.sync.dma_start(out=wk1t[:], in_=Wk[1])
                    wk[1] = wk1t
                if sb == 3:
                    nc.sync.dma_start(out=dkb[:], in_=DK[:, :])
                    kin0 = inp.tile([128, 8, 512], bf16, tag="inT")
                    nc.sync.dma_start(out=kin0[:], in_=kT[0])
                    kits[0] = kin0
                for h in range(2):
                    ps = []
                    for _kc in range(4):
                        pst = pps.tile([128, 512], f32, tag="pp")
                        ps.append(pst)
                    for din in range(8):
                        for kc in range(4):
                            nc.tensor.matmul(
                                ps[kc][:], it[:, din, kc * 128:(kc + 1) * 128],
                                wv0[:, din] if h == 0 else wv1[:, din],
                                start=(din == 0), stop=(din == 7))
                    for kc in range(4):
                        nc.vector.tensor_scalar_mul(
                            v1[:, sb * 4 + kc, h * 512:(h + 1) * 512], ps[kc][:], 1.0)
            wvp.release()

            # =============== phase K: k1T = Wk^T kT (+ fused correction) ===========
            its = kits
            wqp = tc.alloc_tile_pool(name="wqp", bufs=1, side="right")
            qits = [None, None]
            for sb in range(4):
                it = its[sb]
                if sb < 3 and its[sb + 1] is None:
                    kin = inp.tile([128, 8, 512], bf16, tag="inT", name=f"kin{sb+1}")
                    nc.sync.dma_start(out=kin[:], in_=kT[sb + 1])
                    its[sb + 1] = kin
                if sb == 1:
                    wq0t = wqp.tile([128, 4, 8, 128], fp16, tag="wq0")
                    nc.sync.dma_start(out=wq0t[:], in_=Wq[0])
                    wq[0] = wq0t
                if sb == 2:
                    nc.sync.dma_start(out=dqb[:], in_=DQ[:, :])
                    wq1t = wqp.tile([128, 4, 8, 128], fp16, tag="wq1")
                    nc.sync.dma_start(out=wq1t[:], in_=Wq[1])
                    wq[1] = wq1t
                if sb == 3:
                    qin0 = inp.tile([128, 8, 512], bf16, tag="inT")
                    nc.sync.dma_start(out=qin0[:], in_=qT[0])
                    qits[0] = qin0
                for dout in range(8):
                    ps = pps.tile([128, 512], f32, tag="pp")
                    for din in range(8):
                        nc.tensor.matmul(
                            ps[:], wk[dout // 4][:, dout % 4, din, :],
                            it[:, din, :], start=(din == 0), stop=(din == 7))
                    # k1 = d*c + psum, with a single fp32r rounding
                    nc.vector.scalar_tensor_tensor(
                        k1T[:, dout, sb * 512:(sb + 1) * 512],
                        dkb[:, sb * 512:(sb + 1) * 512],
                        ck[:, dout:dout + 1], ps[:],
                        op0=ALU.mult, op1=ALU.add)
            wkp.release()

            # ====== phase Q: q1T = Wq^T qT (+ correction; 1/32 folded into exp) ====
            qin1 = inp.tile([128, 8, 512], bf16, tag="inT")
            nc.sync.dma_start(out=qin1[:], in_=qT[1])
            its = [qits[0], qin1]
            for wh in range(2):
                w = wq[wh]
                for sb in range(2):
                    for d4 in range(4):
                        dout = wh * 4 + d4
                        ps = pps.tile([128, 512], f32, tag="pp")
                        for din in range(8):
                            nc.tensor.matmul(
                                ps[:], w[:, d4, din, :],
                                its[sb][:, din, :], start=(din == 0), stop=(din == 7))
                        nc.vector.scalar_tensor_tensor(
                            q1T[:, dout, sb * 512:(sb + 1) * 512],
                            dqb[:, sb * 512:(sb + 1) * 512],
                            cq[:, dout:dout + 1], ps[:],
                            op0=ALU.mult, op1=ALU.add)
            wqp.release()
            inp.release()
            pps.release()

            # ---- attention, one 128-row query tile per slot ----
            with (
                tc.tile_pool(name="lgp", bufs=3) as lgp,
                tc.tile_pool(name="ppp", bufs=3) as ppp,
                tc.tile_pool(name="otp", bufs=2) as otp,
                tc.tile_pool(name="small", bufs=4) as small,
                tc.tile_pool(name="qkps", bufs=3, space="PSUM") as qkps,
                tc.tile_pool(name="tpps", bufs=2, space="PSUM") as tpps,
                tc.tile_pool(name="svps", bufs=2, space="PSUM") as svps,
            ):
                for s in [2, 3, 4, 5, 6, 7, 1, 0]:
                    E = EXT[s]                # extent in 128-chunks
                    L = E * 128               # extent in keys
                    nb5, nb2 = NB512[s], NB256[s]
                    nb = nb5 + nb2
                    logits = lgp.tile([128, 2048], f32, tag="lg")
                    for b in range(nb):
                        n = 512 if b < nb5 else 256
                        qk = qkps.tile([128, 512], f32, tag="qk")
                        for din in range(8):
                            nc.tensor.matmul(
                                qk[:, :n],
                                q1T[:, din, s * 128:(s + 1) * 128],
                                k1T[:, din, b * 512:b * 512 + n],
                                start=(din == 0), stop=(din == 7))
                        if b == nb - 1:
                            # last block: additive mask on the final 256 cols
                            if n == 512:
                                nc.vector.tensor_scalar_mul(
                                    logits[:, b * 512:b * 512 + 256], qk[:, 0:256], 1.0)
                            nc.vector.tensor_tensor(
                                logits[:, L - 256:L], qk[:, n - 256:n], mb[:],
                                op=ALU.add)
                        else:
                            nc.vector.tensor_scalar_mul(
                                logits[:, b * 512:(b + 1) * 512], qk[:], 1.0)
                    negmax = small.tile([128, 1], f32, tag="negmax")
                    nc.vector.tensor_reduce(
                        negmax[:], logits[:, :L], axis=mybir.AxisListType.X,
                        op=ALU.max, negate=True)
                    # exp(logits - max) + per-block row sums (1/32 is folded
                    # into the q scaling host-side)
                    probs = ppp.tile([128, 16, 128], bf16, tag="probs")
                    p2 = probs[:].rearrange("p a b -> p (a b)")
                    sums = small.tile([128, 4], f32, tag="sums")
                    for b in range(nb):
                        n = 512 if b < nb5 else 256
                        nc.scalar.activation(
                            p2[:, b * 512:b * 512 + n], logits[:, b * 512:b * 512 + n],
                            AF.Exp, bias=negmax[:, 0:1], scale=1.0,
                            accum_out=sums[:, b:b + 1])
                    total = small.tile([128, 1], f32, tag="total")
                    nc.vector.tensor_reduce(
                        total[:], sums[:, :nb], axis=mybir.AxisListType.X,
                        op=ALU.add)
                    recip = small.tile([128, 1], f32, tag="recip")
                    nc.vector.reciprocal(recip[:], total[:])
                    # transpose probs: DMA xbar for big slots, PE for small
                    pT = ppp.tile([128, 16, 128], bf16, tag="pT")
                    if E >= 10:
                        nc.sync.dma_start_transpose(
                            out=pT[:, 0:E, :], in_=p2[:, 0:L])
                    else:
                        for j in range(E):
                            tp = tpps.tile([128, 128], bf16, tag="tp")
                            nc.tensor.transpose(tp[:], probs[:, j, :], ident[:])
                            nc.vector.tensor_scalar_mul(pT[:, j, :], tp[:], 1.0)
                    # SV: out[q, dv] = sum_j pT[j].T @ v1[j, dv]
                    ot = otp.tile([128, D], f32, tag="ot")
                    sv0 = svps.tile([128, 512], f32, tag="sv")
                    sv1 = svps.tile([128, 512], f32, tag="sv")
                    for j in range(E):
                        nc.tensor.matmul(sv0[:], pT[:, j, :], v1[:, j, 0:512],
                                         start=(j == 0), stop=(j == E - 1))
                        nc.tensor.matmul(sv1[:], pT[:, j, :], v1[:, j, 512:1024],
                                         start=(j == 0), stop=(j == E - 1))
                    # normalize by 1/rowsum during evacuation (vector, so the
                    # scalar queue stays free for the next slot's exp)
                    nc.vector.tensor_scalar_mul(ot[:, 0:512], sv0[:], recip[:, 0:1])
                    nc.vector.tensor_scalar_mul(ot[:, 512:1024], sv1[:], recip[:, 0:1])
                    nc.sync.dma_start(out=O[s * 128:(s + 1) * 128, 0:512],
                                      in_=ot[:, 0:512])
                    nc.gpsimd.dma_start(out=O[s * 128:(s + 1) * 128, 512:1024],
                                        in_=ot[:, 512:1024])
            corrK.release()
            warm_p.release()
            consts.release()
    nc.finalize()
    return nc


_NC_CACHE = []


def kernel(q, k, v, mask, W_q, W_k, W_v):
    q = np.asarray(q, dtype=np.float32)
    k = np.asarray(k, dtype=np.float32)
    v = np.asarray(v, dtype=np.float32)
    W_q = np.asarray(W_q, dtype=np.float32)
    W_k = np.asarray(W_k, dtype=np.float32)
    W_v = np.asarray(W_v, dtype=np.float32)

    if not _NC_CACHE:
        _NC_CACHE.append(build())
    nc = _NC_CACHE[0]

    # host-side rank-1 bf16 correction factors
    # d[s] = sum_din (x[din, s] - bf16(x)[din, s]); c[dout] = colmean(W~)
    ckm = W_k.astype(np.float16).astype(np.float64).mean(axis=0).astype(np.float32)
    cqm = W_q.astype(np.float16).astype(np.float64).mean(axis=0).astype(np.float32)
    CKa = np.ascontiguousarray(ckm.reshape(8, 128).T)                   # [128, 8]
    CQa = np.ascontiguousarray(cqm.reshape(8, 128).T)
    WkH = relay_w(W_k)
    WqH = relay_w(W_q)
    WvH = relay_wv(W_v)
    # additive mask tiles: [128, 256] covering the last two 128-chunks
    tri = np.where(np.arange(128)[:, None] >= np.arange(128)[None, :],
                   np.float32(0.0), np.float32(NEG))                    # [128,128]
    MB0 = np.concatenate([np.zeros((128, 128), np.float32), tri], axis=1)
    MB1 = np.concatenate([tri, np.full((128, 128), NEG, np.float32)], axis=1)
    kH, vH, dkH = {}, {}, {}
    for b in range(B):
        kH[b] = relay(k[b].T, 4)
        vH[b] = relay(v[b].T, 4)
        dkH[b] = (k[b] - bf16r(k[b])).sum(axis=1, dtype=np.float64).astype(np.float32)

    row_sets = []
    in_maps = []
    for c in range(8):
        b, cls = c // 2, c % 2
        rows = np.concatenate([np.arange(128 * t, 128 * (t + 1)) for t in TILES[cls]])
        row_sets.append((b, rows))
        qb = q[b][rows, :] * SCALE     # fold 1/sqrt(D) into q
        dq = (qb - bf16r(qb)).sum(axis=1, dtype=np.float64).astype(np.float32)  # [SH]
        in_maps.append({
            "qT": relay(qb.T, 2),
            "kT": kH[b],
            "vT": vH[b],
            "Wq": WqH, "Wk": WkH, "Wv": WvH,
            "MB": MB0 if cls == 0 else MB1,
            "DK": np.ascontiguousarray(np.broadcast_to(dkH[b], (128, S))),
            "DQ": np.ascontiguousarray(np.broadcast_to(dq, (128, SH))),
            "CK": CKa, "CQ": CQa,
        })

    res = run_bass_kernel_spmd(nc, in_maps, core_ids=list(range(8)))

    out = np.empty((B, S, D), dtype=np.float32)
    for c in range(8):
        b, rows = row_sets[c]
        out[b][rows, :] = res.results[c]["O"]
    return out


# revision 15
# speedup vs baseline: 1.0172x; 1.0172x over previous
"""Causal dot-product attention (B=4, S=2048, D=1024) on 8 TRN2 NeuronCores.

v3: bf16 inputs (kT/qT/vT) + bf16 Wv with rank-1 bf16-rounding correction for
K/Q projections (Wk/Wq stay f32r); phase order V->K->Q so the startup
transient needs only ~1.5MB before the PE saturates; PE p-state warm-up
matmuls during the DMA fill; mask via a per-core [128,256] additive tile
fused into the last logits block (no 2MB mask DMA); probs transposed on the
DMA xbar for big slots / PE for small slots; attention slots in descending
extent order.

Sharding: batch x query-tile-class as v2. Core c handles batch c//2; slot s
of class 0 gets tile 15-2s (extent 16-2s chunks), class 1 tile 14-2s
(true extent 15-2s, padded to 16-2s; the pad + diagonal are masked by the
MB tile data, keeping one SPMD program).

Numerics: projections contract bf16 inputs against f32r (K/Q) or bf16 (V)
weights at full PE speed; the dominant rank-1 rounding error
rowsum(x - bf16(x)) (x) colmean(W~) is corrected during PSUM evacuation
(fused scalar_tensor_tensor). QK runs f32r q1/k1. 1/sqrt(D) is applied in
the exp activation; logits' last 256 columns get the additive mask during
evacuation.
"""
import numpy as np
import ml_dtypes
import concourse.bass as bass
import concourse.mybir as mybir
from concourse import bacc
from concourse.tile import TileContext
from concourse.bass_utils import run_bass_kernel_spmd
from concourse.masks import make_identity

f32 = mybir.dt.float32
f32r = mybir.dt.float32r
bf16 = mybir.dt.bfloat16
fp16 = mybir.dt.float16
AF = mybir.ActivationFunctionType
ALU = mybir.AluOpType

B, S, D = 4, 2048, 1024
SH = 1024                  # query rows per core
EXT = [16, 14, 12, 10, 8, 6, 4, 2]        # key extent per slot, 128-chunks
NB512 = [e // 4 for e in EXT]             # full 512 blocks
NB256 = [(e % 4) // 2 for e in EXT]       # trailing 256 block (0 or 1)
TILES = [[15, 13, 11, 9, 7, 5, 3, 1], [14, 12, 10, 8, 6, 4, 2, 0]]
SCALE = 1.0 / 32.0
NEG = -float(2 ** 30)
NWARM = 16


def rne11(x):
    """Bit-exact f32r rounding: RNE to 11 mantissa bits."""
    b = np.asarray(x, dtype=np.float32).view(np.uint32).astype(np.uint64)
    half = np.uint64(1 << 11)
    lsb = (b >> np.uint64(12)) & np.uint64(1)
    b2 = ((b + half - np.uint64(1) + lsb) >> np.uint64(12)) << np.uint64(12)
    return b2.astype(np.uint32).view(np.float32)


def bf16r(x):
    """Round f32 -> bf16 grid (RNE), returned as f32."""
    return np.asarray(x, dtype=np.float32).astype(ml_dtypes.bfloat16).astype(np.float32)


def relay(xT, nsb):
    """[D, N] -> [nsb, 128, 8, N//nsb] per-partition-contiguous, bf16."""
    Dn, N = xT.shape
    w = N // nsb
    out = np.asarray(xT, np.float32).reshape(8, 128, nsb, w).transpose(2, 1, 0, 3)
    return np.ascontiguousarray(out.astype(ml_dtypes.bfloat16))


def relay_w(W):
    """[D, D] fp16 -> [2, 128, 4, 8, 128]: [half][p][d4][din][dout]."""
    out = np.asarray(W, np.float32).reshape(8, 128, 2, 4, 128).transpose(2, 1, 3, 0, 4)
    return np.ascontiguousarray(out.astype(np.float16))


def relay_wv(W):
    """[D, D] bf16 -> [2, 128, 8, 4, 128]: [half][p][din][d4][dout]."""
    out = np.asarray(W, np.float32).reshape(8, 128, 2, 4, 128).transpose(2, 1, 0, 3, 4)
    return np.ascontiguousarray(out.astype(ml_dtypes.bfloat16))


def build():
    nc = bacc.Bacc()
    qT = nc.dram_tensor("qT", [2, 128, 8, 512], bf16, kind="ExternalInput")
    kT = nc.dram_tensor("kT", [4, 128, 8, 512], bf16, kind="ExternalInput")
    vT = nc.dram_tensor("vT", [4, 128, 8, 512], bf16, kind="ExternalInput")
    Wq = nc.dram_tensor("Wq", [2, 128, 4, 8, 128], fp16, kind="ExternalInput")
    Wk = nc.dram_tensor("Wk", [2, 128, 4, 8, 128], fp16, kind="ExternalInput")
    Wv = nc.dram_tensor("Wv", [2, 128, 8, 4, 128], bf16, kind="ExternalInput")
    MB = nc.dram_tensor("MB", [128, 256], f32, kind="ExternalInput")
    DK = nc.dram_tensor("DK", [128, S], f32, kind="ExternalInput")
    DQ = nc.dram_tensor("DQ", [128, SH], f32, kind="ExternalInput")
    CK = nc.dram_tensor("CK", [128, 8], f32, kind="ExternalInput")
    CQ = nc.dram_tensor("CQ", [128, 8], f32, kind="ExternalInput")
    O = nc.dram_tensor("O", [SH, D], f32, kind="ExternalOutput")

    with TileContext(nc) as tc:
        with tc.tile_pool(name="pers", bufs=1) as pers:
            k1T = pers.tile([128, 8, S], f32r, tag="k1T")      # 64 KB/part
            v1 = pers.tile([128, 16, D], bf16, tag="v1")       # 32 KB/part
            q1T = pers.tile([128, 8, SH], f32r, tag="q1T")     # 32 KB/part

            consts = tc.alloc_tile_pool(name="consts", bufs=1, side="left")
            warm_p = tc.alloc_tile_pool(name="warm", bufs=1, side="left")
            corrK = tc.alloc_tile_pool(name="corrK", bufs=1, side="left")
            inp = tc.alloc_tile_pool(name="inp", bufs=3, side="left")
            wvp = tc.alloc_tile_pool(name="wvp", bufs=1, side="right")
            pps = tc.alloc_tile_pool(name="pps", bufs=8, space="PSUM")

            ident = consts.tile([128, 128], bf16, tag="ident")
            mb = consts.tile([128, 256], f32, tag="mb")
            ck = consts.tile([128, 8], f32, tag="ck")
            cq = consts.tile([128, 8], f32, tag="cq")
            dkb = corrK.tile([128, S], f32, tag="dkb")          # 8 KB/part
            dqb = consts.tile([128, SH], f32, tag="dqb")        # 4 KB/part

            # ---- PE p-state warm-up: run scratch matmuls while DMAs fill ----
            scr = warm_p.tile([128, 512], bf16, tag="scr")
            nc.gpsimd.memset(scr[:], 0.0)
            wps = pps.tile([128, 512], f32, tag="pp")
            for _ in range(NWARM):
                nc.tensor.matmul(wps[:, 0:256], scr[:, 0:128], scr[:, 0:256],
                                 start=True, stop=True)

            # =============== phase V: v1 = vT^T Wv (bf16, no correction) ===========
            # startup-critical stream: wv half-pieces on gpsimd, vT0 on sync
            wv0 = wvp.tile([128, 8, 4, 128], bf16, tag="wv0")
            wv1 = wvp.tile([128, 8, 4, 128], bf16, tag="wv1")
            it0 = inp.tile([128, 8, 512], bf16, tag="inT")
            # single sync-queue stream in consumption order: one queue with
            # large pieces sustains ~335GB/s; splitting across queues is slower
            nc.sync.dma_start(out=wv0[:, 0:2], in_=Wv[0, :, 0:2])
            nc.sync.dma_start(out=it0[:, 0:2, :], in_=vT[0, :, 0:2])
            nc.sync.dma_start(out=wv0[:, 2:8], in_=Wv[0, :, 2:8])
            nc.sync.dma_start(out=it0[:, 2:8, :], in_=vT[0, :, 2:8])
            nc.sync.dma_start(out=wv1[:], in_=Wv[1])
            nc.scalar.dma_start(out=ck[:], in_=CK[:, :])
            nc.scalar.dma_start(out=cq[:], in_=CQ[:, :])
            nc.scalar.dma_start(out=mb[:], in_=MB[:, :])
            make_identity(nc, ident[:])
            wk = [None, None]
            wq = [None, None]
            wkp = tc.alloc_tile_pool(name="wkp", bufs=1, side="left")

            its = [it0, None, None, None]
            kits = [None, None, None, None]
            for sb in range(4):
                it = its[sb]
                # prefetch on the single sync stream, consumption order
                if sb < 3:
                    nx = inp.tile([128, 8, 512], bf16, tag="inT", name=f"vin{sb+1}")
                    nc.sync.dma_start(out=nx[:], in_=vT[sb + 1])
                    its[sb + 1] = nx
                if sb == 1:
                    wk0t = wkp.tile([128, 4, 8, 128], fp16, tag="wk0")
                    nc.sync.dma_start(out=wk0t[:], in_=Wk[0])
                    wk[0] = wk0t
                if sb == 2:
                    wk1t = wkp.tile([128, 4, 8, 128], fp16, tag="wk1")
                    nc.sync.dma_start(out=wk1t[:], in_=Wk[1])
                    wk[1] = wk1t
                if sb == 3:
                    nc.sync.dma_start(out=dkb[:], in_=DK[:, :])
                    kin0 = inp.tile([128, 8, 512], bf16, tag="inT")
                    nc.sync.dma_start(out=kin0[:], in_=kT[0])
                    kits[0] = kin0
                for h in range(2):
                    ps = []
                    for _kc in range(4):
                        pst = pps.tile([128, 512], f32, tag="pp")
                        ps.append(pst)
                    for din in range(8):
                        for kc in range(4):
                            nc.tensor.matmul(
                                ps[kc][:], it[:, din, kc * 128:(kc + 1) * 128],
                                wv0[:, din] if h == 0 else wv1[:, din],
                                start=(din == 0), stop=(din == 7))
                    for kc in range(4):
                        nc.vector.tensor_scalar_mul(
                            v1[:, sb * 4 + kc, h * 512:(h + 1) * 512], ps[kc][:], 1.0)
            wvp.release()

            # =============== phase K: k1T = Wk^T kT (+ fused correction) ===========
            its = kits
            wqp = tc.alloc_tile_pool(name="wqp", bufs=1, side="right")
            qits = [None, None]
            for sb in range(4):
                it = its[sb]
                if sb < 3 and its[sb + 1] is None:
                    kin = inp.tile([128, 8, 512], bf16, tag="inT", name=f"kin{sb+1}")
                    nc.sync.dma_start(out=kin[:], in_=kT[sb + 1])
                    its[sb + 1] = kin
                if sb == 1:
                    wq0t = wqp.tile([128, 4, 8, 128], fp16, tag="wq0")
                    nc.sync.dma_start(out=wq0t[:], in_=Wq[0])
                    wq[0] = wq0t
                if sb == 2:
                    nc.sync.dma_start(out=dqb[:], in_=DQ[:, :])
                    wq1t = wqp.tile([128, 4, 8, 128], fp16, tag="wq1")
                    nc.sync.dma_start(out=wq1t[:], in_=Wq[1])
                    wq[1] = wq1t
                if sb == 3:
                    qin0 = inp.tile([128, 8, 512], bf16, tag="inT")
                    nc.sync.dma_start(out=qin0[:], in_=qT[0])
                    qits[0] = qin0
                for dout in range(8):
                    ps = pps.tile([128, 512], f32, tag="pp")
                    for din in range(8):
                        nc.tensor.matmul(
                            ps[:], wk[dout // 4][:, dout % 4, din, :],
                            it[:, din, :], start=(din == 0), stop=(din == 7))
                    # k1 = d*c + psum, with a single fp32r rounding
                    nc.vector.scalar_tensor_tensor(
                        k1T[:, dout, sb * 512:(sb + 1) * 512],
                        dkb[:, sb * 512:(sb + 1) * 512],
                        ck[:, dout:dout + 1], ps[:],
                        op0=ALU.mult, op1=ALU.add)
            wkp.release()

            # ====== phase Q: q1T = Wq^T qT (+ correction; 1/32 folded into exp) ====
            qin1 = inp.tile([128, 8, 512], bf16, tag="inT")
            nc.sync.dma_start(out=qin1[:], in_=qT[1])
            its = [qits[0], qin1]
            for wh in range(2):
                w = wq[wh]
                for sb in range(2):
                    for d4 in range(4):
                        dout = wh * 4 + d4
                        ps = pps.tile([128, 512], f32, tag="pp")
                        for din in range(8):
                            nc.tensor.matmul(
                                ps[:], w[:, d4, din, :],
                                its[sb][:, din, :], start=(din == 0), stop=(din == 7))
                        nc.vector.scalar_tensor_tensor(
                            q1T[:, dout, sb * 512:(sb + 1) * 512],
                            dqb[:, sb * 512:(sb + 1) * 512],
                            cq[:, dout:dout + 1], ps[:],
                            op0=ALU.mult, op1=ALU.add)
            wqp.release()
            inp.release()
            pps.release()

            # ---- attention, one 128-row query tile per slot ----
            with (
                tc.tile_pool(name="lgp", bufs=3) as lgp,
                tc.tile_pool(name="ppp", bufs=3) as ppp,
                tc.tile_pool(name="otp", bufs=2) as otp,
                tc.tile_pool(name="small", bufs=4) as small,
                tc.tile_pool(name="qkps", bufs=3, space="PSUM") as qkps,
                tc.tile_pool(name="tpps", bufs=2, space="PSUM") as tpps,
                tc.tile_pool(name="svps", bufs=2, space="PSUM") as svps,
            ):
                for s in [2, 3, 4, 5, 6, 7, 0, 1]:
                    E = EXT[s]                # extent in 128-chunks
                    L = E * 128               # extent in keys
                    nb5, nb2 = NB512[s], NB256[s]
                    nb = nb5 + nb2
                    logits = lgp.tile([128, 2048], f32, tag="lg")
                    bmax = small.tile([128, 4], f32, tag="bmax")
                    for b in range(nb):
                        n = 512 if b < nb5 else 256
                        qk = qkps.tile([128, 512], f32, tag="qk")
                        for din in range(8):
                            nc.tensor.matmul(
                                qk[:, :n],
                                q1T[:, din, s * 128:(s + 1) * 128],
                                k1T[:, din, b * 512:b * 512 + n],
                                start=(din == 0), stop=(din == 7))
                        if b == nb - 1:
                            # last block: additive mask on the final 256 cols
                            if n == 512:
                                nc.vector.tensor_scalar_mul(
                                    logits[:, b * 512:b * 512 + 256], qk[:, 0:256], 1.0)
                            nc.vector.tensor_tensor(
                                logits[:, L - 256:L], qk[:, n - 256:n], mb[:],
                                op=ALU.add)
                        else:
                            nc.vector.tensor_scalar_mul(
                                logits[:, b * 512:(b + 1) * 512], qk[:], 1.0)
                        # per-block max, overlapped with the next block's QK
                        nc.vector.tensor_reduce(
                            bmax[:, b:b + 1], logits[:, b * 512:b * 512 + n],
                            axis=mybir.AxisListType.X, op=ALU.max)
                    negmax = small.tile([128, 1], f32, tag="negmax")
                    nc.vector.tensor_reduce(
                        negmax[:], bmax[:, :nb], axis=mybir.AxisListType.X,
                        op=ALU.max, negate=True)
                    # exp(logits - max) + per-block row sums (1/32 is folded
                    # into the q scaling host-side)
                    probs = ppp.tile([128, 16, 128], bf16, tag="probs")
                    p2 = probs[:].rearrange("p a b -> p (a b)")
                    sums = small.tile([128, 4], f32, tag="sums")
                    for b in range(nb):
                        n = 512 if b < nb5 else 256
                        nc.scalar.activation(
                            p2[:, b * 512:b * 512 + n], logits[:, b * 512:b * 512 + n],
                            AF.Exp, bias=negmax[:, 0:1], scale=1.0,
                            accum_out=sums[:, b:b + 1])
                    total = small.tile([128, 1], f32, tag="total")
                    nc.vector.tensor_reduce(
                        total[:], sums[:, :nb], axis=mybir.AxisListType.X,
                        op=ALU.add)
                    recip = small.tile([128, 1], f32, tag="recip")
                    nc.vector.reciprocal(recip[:], total[:])
                    # transpose probs: DMA xbar for big slots, PE for small
                    pT = ppp.tile([128, 16, 128], bf16, tag="pT")
                    if E >= 10:
                        h1 = 4 * ((nb5 + 1) // 2)
                        nc.sync.dma_start_transpose(
                            out=pT[:, 0:h1, :], in_=p2[:, 0:h1 * 128])
                        nc.sync.dma_start_transpose(
                            out=pT[:, h1:E, :], in_=p2[:, h1 * 128:L])
                    else:
                        for j in range(E):
                            tp = tpps.tile([128, 128], bf16, tag="tp")
                            nc.tensor.transpose(tp[:], probs[:, j, :], ident[:])
                            nc.vector.tensor_scalar_mul(pT[:, j, :], tp[:], 1.0)
                    # SV: out[q, dv] = sum_j pT[j].T @ v1[j, dv]
                    ot = otp.tile([128, D], f32, tag="ot")
                    sv0 = svps.tile([128, 512], f32, tag="sv")
                    sv1 = svps.tile([128, 512], f32, tag="sv")
                    for j in range(E):
                        nc.tensor.matmul(sv0[:], pT[:, j, :], v1[:, j, 0:512],
                                         start=(j == 0), stop=(j == E - 1))
                        nc.tensor.matmul(sv1[:], pT[:, j, :], v1[:, j, 512:1024],
                                         start=(j == 0), stop=(j == E - 1))
                    # normalize by 1/rowsum during evacuation (vector, so the
                    # scalar queue stays free for the next slot's exp)
                    nc.vector.tensor_scalar_mul(ot[:, 0:512], sv0[:], recip[:, 0:1])
                    nc.vector.tensor_scalar_mul(ot[:, 512:1024], sv1[:], recip[:, 0:1])
                    nc.sync.dma_start(out=O[s * 128:(s + 1) * 128, 0:512],
                                      in_=ot[:, 0:512])
                    nc.gpsimd.dma_start(out=O[s * 128:(s + 1) * 128, 512:1024],
                                        in_=ot[:, 512:1024])
            corrK.release()
            warm_p.release()
            consts.release()
    nc.finalize()
    return nc


_NC_CACHE = []


def kernel(q, k, v, mask, W_q, W_k, W_v):
    q = np.asarray(q, dtype=np.float32)
    k = np.asarray(k, dtype=np.float32)
    v = np.asarray(v, dtype=np.float32)
    W_q = np.asarray(W_q, dtype=np.float32)
    W_k = np.asarray(W_k, dtype=np.float32)
    W_v = np.asarray(W_v, dtype=np.float32)

    if not _NC_CACHE:
        _NC_CACHE.append(build())
    nc = _NC_CACHE[0]

    # host-side rank-1 bf16 correction factors
    # d[s] = sum_din (x[din, s] - bf16(x)[din, s]); c[dout] = colmean(W~)
    ckm = W_k.astype(np.float16).astype(np.float64).mean(axis=0).astype(np.float32)
    cqm = W_q.astype(np.float16).astype(np.float64).mean(axis=0).astype(np.float32)
    CKa = np.ascontiguousarray(ckm.reshape(8, 128).T)                   # [128, 8]
    CQa = np.ascontiguousarray(cqm.reshape(8, 128).T)
    WkH = relay_w(W_k)
    WqH = relay_w(W_q)
    WvH = relay_wv(W_v)
    # additive mask tiles: [128, 256] covering the last two 128-chunks
    tri = np.where(np.arange(128)[:, None] >= np.arange(128)[None, :],
                   np.float32(0.0), np.float32(NEG))                    # [128,128]
    MB0 = np.concatenate([np.zeros((128, 128), np.float32), tri], axis=1)
    MB1 = np.concatenate([tri, np.full((128, 128), NEG, np.float32)], axis=1)
    kH, vH, dkH = {}, {}, {}
    for b in range(B):
        kH[b] = relay(k[b].T, 4)
        vH[b] = relay(v[b].T, 4)
        dkH[b] = (k[b] - bf16r(k[b])).sum(axis=1, dtype=np.float64).astype(np.float32)

    row_sets = []
    in_maps = []
    for c in range(8):
        b, cls = c // 2, c % 2
        rows = np.concatenate([np.arange(128 * t, 128 * (t + 1)) for t in TILES[cls]])
        row_sets.append((b, rows))
        qb = q[b][rows, :] * SCALE     # fold 1/sqrt(D) into q
        dq = (qb - bf16r(qb)).sum(axis=1, dtype=np.float64).astype(np.float32)  # [SH]
        in_maps.append({
            "qT": relay(qb.T, 2),
            "kT": kH[b],
            "vT": vH[b],
            "Wq": WqH, "Wk": WkH, "Wv": WvH,
            "MB": MB0 if cls == 0 else MB1,
            "DK": np.ascontiguousarray(np.broadcast_to(dkH[b], (128, S))),
            "DQ": np.ascontiguousarray(np.broadcast_to(dq, (128, SH))),
            "CK": CKa, "CQ": CQa,
        })

    res = run_bass_kernel_spmd(nc, in_maps, core_ids=list(range(8)))

    out = np.empty((B, S, D), dtype=np.float32)
    for c in range(8):
        b, rows = row_sets[c]
        out[b][rows, :] = res.results[c]["O"]
    return out


# revision 16
# speedup vs baseline: 1.0176x; 1.0003x over previous
"""Causal dot-product attention (B=4, S=2048, D=1024) on 8 TRN2 NeuronCores.

v3: bf16 inputs (kT/qT/vT) + bf16 Wv with rank-1 bf16-rounding correction for
K/Q projections (Wk/Wq stay f32r); phase order V->K->Q so the startup
transient needs only ~1.5MB before the PE saturates; PE p-state warm-up
matmuls during the DMA fill; mask via a per-core [128,256] additive tile
fused into the last logits block (no 2MB mask DMA); probs transposed on the
DMA xbar for big slots / PE for small slots; attention slots in descending
extent order.

Sharding: batch x query-tile-class as v2. Core c handles batch c//2; slot s
of class 0 gets tile 15-2s (extent 16-2s chunks), class 1 tile 14-2s
(true extent 15-2s, padded to 16-2s; the pad + diagonal are masked by the
MB tile data, keeping one SPMD program).

Numerics: projections contract bf16 inputs against f32r (K/Q) or bf16 (V)
weights at full PE speed; the dominant rank-1 rounding error
rowsum(x - bf16(x)) (x) colmean(W~) is corrected during PSUM evacuation
(fused scalar_tensor_tensor). QK runs f32r q1/k1. 1/sqrt(D) is applied in
the exp activation; logits' last 256 columns get the additive mask during
evacuation.
"""
import numpy as np
import ml_dtypes
import concourse.bass as bass
import concourse.mybir as mybir
from concourse import bacc
from concourse.tile import TileContext
from concourse.bass_utils import run_bass_kernel_spmd
from concourse.masks import make_identity

f32 = mybir.dt.float32
f32r = mybir.dt.float32r
bf16 = mybir.dt.bfloat16
fp16 = mybir.dt.float16
AF = mybir.ActivationFunctionType
ALU = mybir.AluOpType

B, S, D = 4, 2048, 1024
SH = 1024                  # query rows per core
EXT = [16, 14, 12, 10, 8, 6, 4, 2]        # key extent per slot, 128-chunks
NB512 = [e // 4 for e in EXT]             # full 512 blocks
NB256 = [(e % 4) // 2 for e in EXT]       # trailing 256 block (0 or 1)
TILES = [[15, 13, 11, 9, 7, 5, 3, 1], [14, 12, 10, 8, 6, 4, 2, 0]]
SCALE = 1.0 / 32.0
NEG = -float(2 ** 30)
NWARM = 18


def rne11(x):
    """Bit-exact f32r rounding: RNE to 11 mantissa bits."""
    b = np.asarray(x, dtype=np.float32).view(np.uint32).astype(np.uint64)
    half = np.uint64(1 << 11)
    lsb = (b >> np.uint64(12)) & np.uint64(1)
    b2 = ((b + half - np.uint64(1) + lsb) >> np.uint64(12)) << np.uint64(12)
    return b2.astype(np.uint32).view(np.float32)


def bf16r(x):
    """Round f32 -> bf16 grid (RNE), returned as f32."""
    return np.asarray(x, dtype=np.float32).astype(ml_dtypes.bfloat16).astype(np.float32)


def relay(xT, nsb):
    """[D, N] -> [nsb, 128, 8, N//nsb] per-partition-contiguous, bf16."""
    Dn, N = xT.shape
    w = N // nsb
    out = np.asarray(xT, np.float32).reshape(8, 128, nsb, w).transpose(2, 1, 0, 3)
    return np.ascontiguousarray(out.astype(ml_dtypes.bfloat16))


def relay_w(W):
    """[D, D] fp16 -> [2, 128, 4, 8, 128]: [half][p][d4][din][dout]."""
    out = np.asarray(W, np.float32).reshape(8, 128, 2, 4, 128).transpose(2, 1, 3, 0, 4)
    return np.ascontiguousarray(out.astype(np.float16))


def relay_wv(W):
    """[D, D] bf16 -> [2, 128, 8, 4, 128]: [half][p][din][d4][dout]."""
    out = np.asarray(W, np.float32).reshape(8, 128, 2, 4, 128).transpose(2, 1, 0, 3, 4)
    return np.ascontiguousarray(out.astype(ml_dtypes.bfloat16))


def build():
    nc = bacc.Bacc()
    qT = nc.dram_tensor("qT", [2, 128, 8, 512], bf16, kind="ExternalInput")
    kT = nc.dram_tensor("kT", [4, 128, 8, 512], bf16, kind="ExternalInput")
    vT = nc.dram_tensor("vT", [4, 128, 8, 512], bf16, kind="ExternalInput")
    Wq = nc.dram_tensor("Wq", [2, 128, 4, 8, 128], fp16, kind="ExternalInput")
    Wk = nc.dram_tensor("Wk", [2, 128, 4, 8, 128], fp16, kind="ExternalInput")
    Wv = nc.dram_tensor("Wv", [2, 128, 8, 4, 128], bf16, kind="ExternalInput")
    MB = nc.dram_tensor("MB", [128, 256], f32, kind="ExternalInput")
    DK = nc.dram_tensor("DK", [128, S], f32, kind="ExternalInput")
    DQ = nc.dram_tensor("DQ", [128, SH], f32, kind="ExternalInput")
    CK = nc.dram_tensor("CK", [128, 8], f32, kind="ExternalInput")
    CQ = nc.dram_tensor("CQ", [128, 8], f32, kind="ExternalInput")
    O = nc.dram_tensor("O", [SH, D], f32, kind="ExternalOutput")

    with TileContext(nc) as tc:
        with tc.tile_pool(name="pers", bufs=1) as pers:
            k1T = pers.tile([128, 8, S], f32r, tag="k1T")      # 64 KB/part
            v1 = pers.tile([128, 16, D], bf16, tag="v1")       # 32 KB/part
            q1T = pers.tile([128, 8, SH], f32r, tag="q1T")     # 32 KB/part

            consts = tc.alloc_tile_pool(name="consts", bufs=1, side="left")
            warm_p = tc.alloc_tile_pool(name="warm", bufs=1, side="left")
            corrK = tc.alloc_tile_pool(name="corrK", bufs=1, side="left")
            inp = tc.alloc_tile_pool(name="inp", bufs=3, side="left")
            wvp = tc.alloc_tile_pool(name="wvp", bufs=1, side="right")
            pps = tc.alloc_tile_pool(name="pps", bufs=8, space="PSUM")

            ident = consts.tile([128, 128], bf16, tag="ident")
            mb = consts.tile([128, 256], f32, tag="mb")
            ck = consts.tile([128, 8], f32, tag="ck")
            cq = consts.tile([128, 8], f32, tag="cq")
            dkb = corrK.tile([128, S], f32, tag="dkb")          # 8 KB/part
            dqb = consts.tile([128, SH], f32, tag="dqb")        # 4 KB/part

            # ---- PE p-state warm-up: run scratch matmuls while DMAs fill ----
            scr = warm_p.tile([128, 512], bf16, tag="scr")
            nc.gpsimd.memset(scr[:], 0.0)
            wps = pps.tile([128, 512], f32, tag="pp")
            for _ in range(NWARM):
                nc.tensor.matmul(wps[:, 0:256], scr[:, 0:128], scr[:, 0:256],
                                 start=True, stop=True)

            # =============== phase V: v1 = vT^T Wv (bf16, no correction) ===========
            # startup-critical stream: wv half-pieces on gpsimd, vT0 on sync
            wv0 = wvp.tile([128, 8, 4, 128], bf16, tag="wv0")
            wv1 = wvp.tile([128, 8, 4, 128], bf16, tag="wv1")
            it0 = inp.tile([128, 8, 512], bf16, tag="inT")
            # single sync-queue stream in consumption order: one queue with
            # large pieces sustains ~335GB/s; splitting across queues is slower
            nc.sync.dma_start(out=wv0[:, 0:2], in_=Wv[0, :, 0:2])
            nc.sync.dma_start(out=it0[:, 0:2, :], in_=vT[0, :, 0:2])
            nc.sync.dma_start(out=wv0[:, 2:4], in_=Wv[0, :, 2:4])
            nc.sync.dma_start(out=it0[:, 2:4, :], in_=vT[0, :, 2:4])
            nc.sync.dma_start(out=wv0[:, 4:8], in_=Wv[0, :, 4:8])
            nc.sync.dma_start(out=it0[:, 4:8, :], in_=vT[0, :, 4:8])
            nc.sync.dma_start(out=wv1[:], in_=Wv[1])
            nc.scalar.dma_start(out=ck[:], in_=CK[:, :])
            nc.scalar.dma_start(out=cq[:], in_=CQ[:, :])
            nc.scalar.dma_start(out=mb[:], in_=MB[:, :])
            make_identity(nc, ident[:])
            wk = [None, None]
            wq = [None, None]
            wkp = tc.alloc_tile_pool(name="wkp", bufs=1, side="left")

            its = [it0, None, None, None]
            kits = [None, None, None, None]
            for sb in range(4):
                it = its[sb]
                # prefetch on the single sync stream, consumption order
                if sb < 3:
                    nx = inp.tile([128, 8, 512], bf16, tag="inT", name=f"vin{sb+1}")
                    nc.sync.dma_start(out=nx[:], in_=vT[sb + 1])
                    its[sb + 1] = nx
                if sb == 1:
                    wk0t = wkp.tile([128, 4, 8, 128], fp16, tag="wk0")
                    nc.sync.dma_start(out=wk0t[:], in_=Wk[0])
                    wk[0] = wk0t
                if sb == 2:
                    wk1t = wkp.tile([128, 4, 8, 128], fp16, tag="wk1")
                    nc.sync.dma_start(out=wk1t[:], in_=Wk[1])
                    wk[1] = wk1t
                if sb == 3:
                    nc.sync.dma_start(out=dkb[:], in_=DK[:, :])
                    kin0 = inp.tile([128, 8, 512], bf16, tag="inT")
                    nc.sync.dma_start(out=kin0[:], in_=kT[0])
                    kits[0] = kin0
                for h in range(2):
                    ps = []
                    for _kc in range(4):
                        pst = pps.tile([128, 512], f32, tag="pp")
                        ps.append(pst)
                    for din in range(8):
                        for kc in range(4):
                            nc.tensor.matmul(
                                ps[kc][:], it[:, din, kc * 128:(kc + 1) * 128],
                                wv0[:, din] if h == 0 else wv1[:, din],
                                start=(din == 0), stop=(din == 7))
                    for kc in range(4):
                        nc.vector.tensor_scalar_mul(
                            v1[:, sb * 4 + kc, h * 512:(h + 1) * 512], ps[kc][:], 1.0)
            wvp.release()

            # =============== phase K: k1T = Wk^T kT (+ fused correction) ===========
            its = kits
            wqp = tc.alloc_tile_pool(name="wqp", bufs=1, side="right")
            qits = [None, None]
            for sb in range(4):
                it = its[sb]
                if sb < 3 and its[sb + 1] is None:
                    kin = inp.tile([128, 8, 512], bf16, tag="inT", name=f"kin{sb+1}")
                    nc.sync.dma_start(out=kin[:], in_=kT[sb + 1])
                    its[sb + 1] = kin
                if sb == 1:
                    wq0t = wqp.tile([128, 4, 8, 128], fp16, tag="wq0")
                    nc.sync.dma_start(out=wq0t[:], in_=Wq[0])
                    wq[0] = wq0t
                if sb == 2:
                    nc.sync.dma_start(out=dqb[:], in_=DQ[:, :])
                    wq1t = wqp.tile([128, 4, 8, 128], fp16, tag="wq1")
                    nc.sync.dma_start(out=wq1t[:], in_=Wq[1])
                    wq[1] = wq1t
                if sb == 3:
                    qin0 = inp.tile([128, 8, 512], bf16, tag="inT")
                    nc.sync.dma_start(out=qin0[:], in_=qT[0])
                    qits[0] = qin0
                for dout in range(8):
                    ps = pps.tile([128, 512], f32, tag="pp")
                    for din in range(8):
                        nc.tensor.matmul(
                            ps[:], wk[dout // 4][:, dout % 4, din, :],
                            it[:, din, :], start=(din == 0), stop=(din == 7))
                    # k1 = d*c + psum, with a single fp32r rounding
                    nc.vector.scalar_tensor_tensor(
                        k1T[:, dout, sb * 512:(sb + 1) * 512],
                        dkb[:, sb * 512:(sb + 1) * 512],
                        ck[:, dout:dout + 1], ps[:],
                        op0=ALU.mult, op1=ALU.add)
            wkp.release()

            # ====== phase Q: q1T = Wq^T qT (+ correction; 1/32 folded into exp) ====
            qin1 = inp.tile([128, 8, 512], bf16, tag="inT")
            nc.sync.dma_start(out=qin1[:], in_=qT[1])
            its = [qits[0], qin1]
            for wh in range(2):
                w = wq[wh]
                for sb in range(2):
                    for d4 in range(4):
                        dout = wh * 4 + d4
                        ps = pps.tile([128, 512], f32, tag="pp")
                        for din in range(8):
                            nc.tensor.matmul(
                                ps[:], w[:, d4, din, :],
                                its[sb][:, din, :], start=(din == 0), stop=(din == 7))
                        nc.vector.scalar_tensor_tensor(
                            q1T[:, dout, sb * 512:(sb + 1) * 512],
                            dqb[:, sb * 512:(sb + 1) * 512],
                            cq[:, dout:dout + 1], ps[:],
                            op0=ALU.mult, op1=ALU.add)
            wqp.release()
            inp.release()
            pps.release()

            # ---- attention, one 128-row query tile per slot ----
            with (
                tc.tile_pool(name="lgp", bufs=3) as lgp,
                tc.tile_pool(name="ppp", bufs=3) as ppp,
                tc.tile_pool(name="otp", bufs=2) as otp,
                tc.tile_pool(name="small", bufs=4) as small,
                tc.tile_pool(name="qkps", bufs=3, space="PSUM") as qkps,
                tc.tile_pool(name="tpps", bufs=2, space="PSUM") as tpps,
                tc.tile_pool(name="svps", bufs=2, space="PSUM") as svps,
            ):
                for s in [2, 3, 4, 5, 6, 7, 0, 1]:
                    E = EXT[s]                # extent in 128-chunks
                    L = E * 128               # extent in keys
                    nb5, nb2 = NB512[s], NB256[s]
                    nb = nb5 + nb2
                    logits = lgp.tile([128, 2048], f32, tag="lg")
                    bmax = small.tile([128, 4], f32, tag="bmax")
                    for b in range(nb):
                        n = 512 if b < nb5 else 256
                        qk = qkps.tile([128, 512], f32, tag="qk")
                        for din in range(8):
                            nc.tensor.matmul(
                                qk[:, :n],
                                q1T[:, din, s * 128:(s + 1) * 128],
                                k1T[:, din, b * 512:b * 512 + n],
                                start=(din == 0), stop=(din == 7))
                        if b == nb - 1:
                            # last block: additive mask on the final 256 cols
                            if n == 512:
                                nc.vector.tensor_scalar_mul(
                                    logits[:, b * 512:b * 512 + 256], qk[:, 0:256], 1.0)
                            nc.vector.tensor_tensor(
                                logits[:, L - 256:L], qk[:, n - 256:n], mb[:],
                                op=ALU.add)
                        else:
                            nc.vector.tensor_scalar_mul(
                                logits[:, b * 512:(b + 1) * 512], qk[:], 1.0)
                        # per-block max, overlapped with the next block's QK
                        nc.vector.tensor_reduce(
                            bmax[:, b:b + 1], logits[:, b * 512:b * 512 + n],
                            axis=mybir.AxisListType.X, op=ALU.max)
                    negmax = small.tile([128, 1], f32, tag="negmax")
                    nc.vector.tensor_reduce(
                        negmax[:], bmax[:, :nb], axis=mybir.AxisListType.X,
                        op=ALU.max, negate=True)
                    # exp(logits - max) + per-block row sums (1/32 is folded
                    # into the q scaling host-side)
                    probs = ppp.tile([128, 16, 128], bf16, tag="probs")
                    p2 = probs[:].rearrange("p a b -> p (a b)")
                    sums = small.tile([128, 4], f32, tag="sums")
                    for b in range(nb):
                        n = 512 if b < nb5 else 256
                        nc.scalar.activation(
                            p2[:, b * 512:b * 512 + n], logits[:, b * 512:b * 512 + n],
                            AF.Exp, bias=negmax[:, 0:1], scale=1.0,
                            accum_out=sums[:, b:b + 1])
                    total = small.tile([128, 1], f32, tag="total")
                    nc.vector.tensor_reduce(
                        total[:], sums[:, :nb], axis=mybir.AxisListType.X,
                        op=ALU.add)
                    recip = small.tile([128, 1], f32, tag="recip")
                    nc.vector.reciprocal(recip[:], total[:])
                    # transpose probs: DMA xbar for big slots, PE for small
                    pT = ppp.tile([128, 16, 128], bf16, tag="pT")
                    if E >= 10:
                        h1 = 4 * ((nb5 + 1) // 2)
                        nc.sync.dma_start_transpose(
                            out=pT[:, 0:h1, :], in_=p2[:, 0:h1 * 128])
                        nc.sync.dma_start_transpose(
                            out=pT[:, h1:E, :], in_=p2[:, h1 * 128:L])
                    else:
                        for j in range(E):
                            tp = tpps.tile([128, 128], bf16, tag="tp")
                            nc.tensor.transpose(tp[:], probs[:, j, :], ident[:])
                            nc.vector.tensor_scalar_mul(pT[:, j, :], tp[:], 1.0)
                    # SV: out[q, dv] = sum_j pT[j].T @ v1[j, dv]; two halves
                    # sequentially so the first half's evac+store overlaps the
                    # second half's matmuls
                    ot = otp.tile([128, D], f32, tag="ot")
                    sv0 = svps.tile([128, 512], f32, tag="sv")
                    for j in range(E):
                        nc.tensor.matmul(sv0[:], pT[:, j, :], v1[:, j, 0:512],
                                         start=(j == 0), stop=(j == E - 1))
                    nc.vector.tensor_scalar_mul(ot[:, 0:512], sv0[:], recip[:, 0:1])
                    nc.sync.dma_start(out=O[s * 128:(s + 1) * 128, 0:512],
                                      in_=ot[:, 0:512])
                    sv1 = svps.tile([128, 512], f32, tag="sv")
                    for j in range(E):
                        nc.tensor.matmul(sv1[:], pT[:, j, :], v1[:, j, 512:1024],
                                         start=(j == 0), stop=(j == E - 1))
                    nc.vector.tensor_scalar_mul(ot[:, 512:1024], sv1[:], recip[:, 0:1])
                    nc.gpsimd.dma_start(out=O[s * 128:(s + 1) * 128, 512:1024],
                                        in_=ot[:, 512:1024])
            corrK.release()
            warm_p.release()
            consts.release()
    nc.finalize()
    return nc


_NC_CACHE = []


def kernel(q, k, v, mask, W_q, W_k, W_v):
    q = np.asarray(q, dtype=np.float32)
    k = np.asarray(k, dtype=np.float32)
    v = np.asarray(v, dtype=np.float32)
    W_q = np.asarray(W_q, dtype=np.float32)
    W_k = np.asarray(W_k, dtype=np.float32)
    W_v = np.asarray(W_v, dtype=np.float32)

    if not _NC_CACHE:
        _NC_CACHE.append(build())
    nc = _NC_CACHE[0]

    # host-side rank-1 bf16 correction factors
    # d[s] = sum_din (x[din, s] - bf16(x)[din, s]); c[dout] = colmean(W~)
    ckm = W_k.astype(np.float16).astype(np.float64).mean(axis=0).astype(np.float32)
    cqm = W_q.astype(np.float16).astype(np.float64).mean(axis=0).astype(np.float32)
    CKa = np.ascontiguousarray(ckm.reshape(8, 128).T)                   # [128, 8]
    CQa = np.ascontiguousarray(cqm.reshape(8, 128).T)
    WkH = relay_w(W_k)
    WqH = relay_w(W_q)
    WvH = relay_wv(W_v)
    # additive mask tiles: [128, 256] covering the last two 128-chunks
    tri = np.where(np.arange(128)[:, None] >= np.arange(128)[None, :],
                   np.float32(0.0), np.float32(NEG))                    # [128,128]
    MB0 = np.concatenate([np.zeros((128, 128), np.float32), tri], axis=1)
    MB1 = np.concatenate([tri, np.full((128, 128), NEG, np.float32)], axis=1)
    kH, vH, dkH = {}, {}, {}
    for b in range(B):
        kH[b] = relay(k[b].T, 4)
        vH[b] = relay(v[b].T, 4)
        dkH[b] = (k[b] - bf16r(k[b])).sum(axis=1, dtype=np.float64).astype(np.float32)

    row_sets = []
    in_maps = []
    for c in range(8):
        b, cls = c // 2, c % 2
        rows = np.concatenate([np.arange(128 * t, 128 * (t + 1)) for t in TILES[cls]])
        row_sets.append((b, rows))
        qb = q[b][rows, :] * SCALE     # fold 1/sqrt(D) into q
        dq = (qb - bf16r(qb)).sum(axis=1, dtype=np.float64).astype(np.float32)  # [SH]
        in_maps.append({
            "qT": relay(qb.T, 2),
            "kT": kH[b],
            "vT": vH[b],
            "Wq": WqH, "Wk": WkH, "Wv": WvH,
            "MB": MB0 if cls == 0 else MB1,
            "DK": np.ascontiguousarray(np.broadcast_to(dkH[b], (128, S))),
            "DQ": np.ascontiguousarray(np.broadcast_to(dq, (128, SH))),
            "CK": CKa, "CQ": CQa,
        })

    res = run_bass_kernel_spmd(nc, in_maps, core_ids=list(range(8)))

    out = np.empty((B, S, D), dtype=np.float32)
    for c in range(8):
        b, rows = row_sets[c]
        out[b][rows, :] = res.results[c]["O"]
    return out
